# revision 32
# baseline (speedup 1.0000x reference)
"""CrossAttention Trainium2 Bass kernel (v2: phase-overlapped bf16 schedule).

Problem: y = CrossAttention(x, kv) with the reference's no-transpose q-reshape
quirk, B=8, N=1024, C=768, H=8, D=96.

Strategy: pure data parallelism — batch element b on NeuronCore b. Host
pre-transposes x/kv/weights, converts them to bf16 (halves input DMA and
SBUF; matmul rate is identical to float32r at 1 row/cycle; emulated end-to-end
rel err 0.6% vs the 2e-2 gate). All on-chip intermediates are bf16 with fp32
PSUM accumulation.

The ACT engine's ~68us of softmax-exp is the attention bottleneck, so the
schedule starts S=K^T Q tiles as soon as head 0's K/Q slices exist (~17us)
and spreads the remaining projection waves / PV / output-projection work
between S emissions so PE never starves while ACT drains. The output
projection is split into three DMA-accumulated passes (heads 0-3 / 4-6 / 7)
so the post-PV(7) tail is only a few us.

PSUM: pS 4 banks (S tiles), pPV 2 banks (PV accumulators), pW 2 banks
(projection waves + yproj). Projection waves run as 2-group sub-waves on pW.
"""
import sys
sys.path.insert(0, '/opt/trn_rl_repo')

import numpy as np
import concourse.bass as bass
import concourse.mybir as mybir
import concourse.tile as tile
from concourse.bass_utils import run_bass_kernel_spmd

F32 = mybir.dt.float32
BF16 = mybir.dt.bfloat16
AF = mybir.ActivationFunctionType

B, N, C = 8, 1024, 768
H, D = 8, 96
SCALE = D ** -0.5
NB = N // 128   # 8 n-blocks
CB = C // 128   # 6 c-blocks
HN = H * N      # 8192


def _legalize_waits(nc, max_waits=1):
    """This container's walrus accepts at most one sync-wait command per
    instruction; move excess waits onto preceding NoOps on the same engine."""
    ctr = 0
    for f in nc.m.functions:
        for blk in f.blocks:
            out = []
            changed = False
            for ins in blk.instructions:
                si = ins.sync_info
                waits = list(si.on_wait) if si is not None and si.on_wait else []
                if len(waits) > max_waits:
                    changed = True
                    for w in waits[:-max_waits]:
                        ctr += 1
                        nop = mybir.InstNoOp(name=f"I-wsplit-{ctr}")
                        nop.engine = ins.engine
                        nop.sync_info = mybir.SyncInfo(on_wait=[w], on_update=[])
                        out.append(nop)
                    ins.sync_info = mybir.SyncInfo(
                        on_wait=waits[-max_waits:],
                        on_update=list(si.on_update or []))
                out.append(ins)
            if changed:
                blk.instructions = out
    return ctr


def build_kernel(repeat=1):
    nc = bass.Bass('TRN2', target_bir_lowering=False, debug=False, num_devices=B)

    xT = nc.dram_tensor("xT", [C, N], BF16, kind="ExternalInput").ap()
    kvT = nc.dram_tensor("kvT", [C, N], BF16, kind="ExternalInput").ap()
    WqT = nc.dram_tensor("WqT", [C, C], BF16, kind="ExternalInput").ap()
    WkvT = nc.dram_tensor("WkvT", [C, 2 * C], BF16, kind="ExternalInput").ap()
    WpjT = nc.dram_tensor("WpjT", [C, C], BF16, kind="ExternalInput").ap()
    bias = nc.dram_tensor("bias", [1, C], BF16, kind="ExternalInput").ap()
    y = nc.dram_tensor("y", [N, C], F32, kind="ExternalOutput").ap()
    rs_dram = nc.dram_tensor("rs_scratch", [1, HN], BF16, kind="Internal").ap()
    ri_dram = nc.dram_tensor("ri_scratch", [1, HN], BF16, kind="Internal").ap()

    with tile.TileContext(nc) as tc:
      for _rep in range(repeat):
        with tc.tile_pool(name="persist", bufs=1) as pp, \
             tc.tile_pool(name="norm", bufs=1) as pn, \
             tc.tile_pool(name="ptile", bufs=25) as ppt, \
             tc.tile_pool(name="yout", bufs=3) as py, \
             tc.tile_pool(name="wkv", bufs=1) as pwkv, \
             tc.tile_pool(name="psS", bufs=2, space="PSUM") as pS, \
             tc.tile_pool(name="psPV", bufs=2, space="PSUM") as pPV, \
             tc.tile_pool(name="psW", bufs=1, space="PSUM") as pW:
            QT = pp.tile([D, HN], BF16, tag="QT")
            KT = pp.tile([D, HN], BF16, tag="KT")
            V = [pp.tile([128, H * 97], BF16, tag=f"V{i}", name=f"V{i}")
                 for i in range(NB)]
            Oall = pp.tile([97, HN], BF16, tag="Oall")
            # partial y (heads 0-3) per n-block, bf16 accumulator staging
            Ybuf = [pp.tile([128, C], BF16, tag=f"Yb{i}", name=f"Ybuf{i}")
                    for i in range(NB)]

            ones97f = pn.tile([1, 97], F32, tag="o97f")
            nc.vector.memset(ones97f[:], 1.0)
            ones97 = pn.tile([1, 97], BF16, tag="o97")
            nc.vector.tensor_copy(ones97[:], ones97f[:])

            kvTs = [pwkv.tile([128, N], BF16, tag=f"kv{i}", name=f"kvTs{i}")
                    for i in range(CB)]
            WkvTs = [pwkv.tile([128, 2 * C], BF16, tag=f"Wkv{i}",
                               name=f"WkvTs{i}") for i in range(CB)]

            # ---------------- helpers ----------------
            def q_evac(g, ps):
                r, u = g
                dest = QT[:].rearrange(
                    "p (h j r) -> p h j r", h=H, j=128)[
                    :, 4 * u:4 * (u + 1), :, r:r + 1]
                nc.vector.tensor_copy(dest, ps)

            def k_evac(g, ps):
                h, u = g
                nc.vector.tensor_copy(
                    KT[:, 1024 * h + 512 * u:1024 * h + 512 * (u + 1)], ps)

            def v_evac(g, ps):
                nb, u = g
                dest = V[nb][:].rearrange(
                    "p (h c) -> p h c", h=H)[:, 4 * u:4 * (u + 1), 0:96]
                nc.vector.tensor_copy(dest, ps)

            _sw = [0]

            def subwave_units(groups, lhsT_of, rhs_of, evac, mm_parts, ncols):
                """One 2-group sub-wave on pW -> list of emit closures
                (6 cb-steps + 1 evac). The psum tile is allocated when the
                first closure runs."""
                assert len(groups) == 2
                state = {}

                def step(cb):
                    if cb == 0:
                        _sw[0] += 1
                        t = pW.tile([128, 1024], F32, tag="mm",
                                    name=f"sw{_sw[0]}")
                        state['slots'] = [t[0:mm_parts, 0:ncols],
                                          t[0:mm_parts, 512:512 + ncols]]
                    for g, ps in zip(groups, state['slots']):
                        nc.tensor.matmul(
                            ps, lhsT_of(g, cb), rhs_of(g, cb),
                            start=(cb == 0), stop=(cb == CB - 1))

                def final():
                    for g, ps in zip(groups, state['slots']):
                        evac(g, ps)
                return [lambda cb=cb: step(cb) for cb in range(CB)] + [final]

            def wave_subwaves(groups, *a):
                return [subwave_units(groups[i:i + 2], *a)
                        for i in range(0, len(groups), 2)]

            class BulkStream:
                def __init__(self):
                    self.units = []

                def add(self, subwaves):
                    for sw in subwaves:
                        self.units.extend(sw)

                def emit(self, n=1):
                    for _ in range(n):
                        if self.units:
                            self.units.pop(0)()

                def drain(self):
                    while self.units:
                        self.units.pop(0)()

            P_tiles = {h: [None] * NB for h in range(H)}

            def S_unit(h, kb):
                pt = ppt.tile([128, N], BF16, tag="pt", name=f"P{h}_{kb}")
                ps = pS.tile([128, 1024], F32, tag="s", name=f"s{h}_{kb}")
                for u in range(2):
                    nc.tensor.matmul(
                        ps[:, 512 * u:512 * (u + 1)],
                        KT[:, 1024 * h + 128 * kb:1024 * h + 128 * (kb + 1)],
                        QT[:, 1024 * h + 512 * u:1024 * h + 512 * (u + 1)],
                        start=True, stop=True)
                nc.scalar.activation(pt[:], ps[:], AF.Exp)
                P_tiles[h][kb] = pt

            pv_state = {}

            def PV_unit(h, kb):
                if kb == 0:
                    pv_state[h] = [pPV.tile([97, 512], F32, tag="po",
                                            name=f"po{h}_{u}")
                                   for u in range(2)]
                for u in range(2):
                    nc.tensor.matmul(
                        pv_state[h][u][:],
                        V[kb][:, 97 * h:97 * (h + 1)],
                        P_tiles[h][kb][:, 512 * u:512 * (u + 1)],
                        start=(kb == 0), stop=(kb == NB - 1))
                if kb == NB - 1:
                    for u in range(2):
                        nc.vector.tensor_copy(
                            Oall[:, 1024 * h + 512 * u:
                                 1024 * h + 512 * (u + 1)],
                            pv_state[h][u][:])
                    P_tiles[h] = None

            def emit_norm(h):
                """rowsum -> 1/rowsum broadcast (DMA round trip) ->
                in-place normalize Oall's head-h slice."""
                sl = slice(1024 * h, 1024 * (h + 1))
                nc.sync.dma_start(rs_dram[0:1, sl], Oall[96:97, sl])
                rsh = pn.tile([128, 8], BF16, tag="rs", name=f"rs{h}", bufs=2)
                nc.sync.dma_start(
                    rsh[:],
                    rs_dram[0:1, sl].rearrange("p (a b) -> (p a) b", a=128))
                rih = pn.tile([128, 8], F32, tag="ri", name=f"ri{h}", bufs=2)
                nc.vector.reciprocal(rih[:], rsh[:])
                rirh = pn.tile([128, 8], BF16, tag="rir", name=f"rir{h}",
                               bufs=2)
                nc.vector.tensor_copy(rirh[:], rih[:])
                nc.sync.dma_start(
                    ri_dram[0:1, sl].rearrange("p (a b) -> (p a) b", a=128),
                    rirh[:])
                bch = pn.tile([97, N], BF16, tag="bc", name=f"bc{h}", bufs=2)
                nc.sync.dma_start(
                    bch[:], bass.AP(ri_dram.tensor, 1024 * h, [[0, 97], [1, N]]))
                nc.vector.tensor_mul(Oall[:, sl], Oall[:, sl], bch[:])

            def emit_norm_dve(h):
                """Lowest-latency norm: reciprocal of the rowsum row on DVE
                (single partition), broadcast via K=1 ones matmul on PE.
                No DRAM round trip, no ACT table dependency."""
                sl = slice(1024 * h, 1024 * (h + 1))
                invt = pn.tile([1, N], BF16, tag="invr2", name=f"inv2_{h}",
                               bufs=2)
                with nc.allow_low_precision(reason="1/rowsum broadcast scale"):
                    nc.vector.reciprocal(invt[:], Oall[96:97, sl])
                for u in range(2):
                    bc_ps = pPV.tile([97, 512], F32, tag="po",
                                     name=f"bcd{h}_{u}")
                    nc.tensor.matmul(
                        bc_ps[:], ones97[:],
                        invt[0:1, 512 * u:512 * (u + 1)],
                        start=True, stop=True)
                    ssl = slice(1024 * h + 512 * u, 1024 * h + 512 * (u + 1))
                    nc.vector.tensor_mul(Oall[:, ssl], Oall[:, ssl], bc_ps[:])

            def emit_norm_fast(h):
                """No-DMA tail variant: inv = exp(-ln(rowsum)) on ACT,
                broadcast via K=1 ones matmul on PE."""
                sl = slice(1024 * h, 1024 * (h + 1))
                lnr = pn.tile([1, N], F32, tag="lnx", name=f"lnr{h}")
                nc.scalar.activation(lnr[:], Oall[96:97, sl], AF.Ln)
                invt = pn.tile([1, N], BF16, tag="invr", name=f"invr{h}")
                nc.scalar.activation(invt[:], lnr[:], AF.Exp, scale=-1.0)
                for u in range(2):
                    bc_ps = pPV.tile([97, 512], F32, tag="po",
                                     name=f"bcps{h}_{u}")
                    nc.tensor.matmul(
                        bc_ps[:], ones97[:],
                        invt[0:1, 512 * u:512 * (u + 1)],
                        start=True, stop=True)
                    ssl = slice(1024 * h + 512 * u, 1024 * h + 512 * (u + 1))
                    nc.vector.tensor_mul(Oall[:, ssl], Oall[:, ssl], bc_ps[:])

            # ---------------- phase A+B ----------------
            with tc.tile_pool(name="wproj", bufs=1) as pwp:
              Wp = []
              for h in range(H):
                  rows = 97 if h == H - 1 else 96
                  Wp.append(pwp.tile([rows, C], BF16, tag=f"Wp{h}",
                                     name=f"Wp{h}"))
              with tc.tile_pool(name="wq", bufs=1) as pwq:
                xTs = [pwq.tile([128, N], BF16, tag=f"xT{i}", name=f"xTs{i}")
                       for i in range(CB)]
                WqTs = [pwq.tile([128, C], BF16, tag=f"Wq{i}", name=f"WqTs{i}")
                        for i in range(CB)]

                # DMA issue in consumption order:
                # W1 (Wq + x half0), W2 (kv + Wkv K-half lo), W5 (Wkv V lo),
                # W4 (x half1), W3 (Wkv K-half hi), W6 (Wkv V hi), Wp+bias.
                nc.sync.dma_start(WqTs[0][:, 0:192], WqT[0:128, 0:192])
                nc.sync.dma_start(xTs[0][:, 0:512], xT[0:128, 0:512])
                nc.sync.dma_start(WqTs[0][:, 192:C], WqT[0:128, 192:C])
                for i in range(1, CB):
                    nc.sync.dma_start(WqTs[i][:], WqT[128 * i:128 * (i + 1), :])
                    nc.sync.dma_start(xTs[i][:, 0:512],
                                      xT[128 * i:128 * (i + 1), 0:512])
                for i in range(CB):
                    nc.sync.dma_start(kvTs[i][:], kvT[128 * i:128 * (i + 1), :])
                    nc.sync.dma_start(WkvTs[i][:, 0:384],
                                      WkvT[128 * i:128 * (i + 1), 0:384])
                for i in range(CB):
                    nc.sync.dma_start(WkvTs[i][:, 768:1152],
                                      WkvT[128 * i:128 * (i + 1), 768:1152])
                for i in range(CB):
                    nc.sync.dma_start(xTs[i][:, 512:1024],
                                      xT[128 * i:128 * (i + 1), 512:1024])
                for i in range(CB):
                    nc.sync.dma_start(WkvTs[i][:, 384:768],
                                      WkvT[128 * i:128 * (i + 1), 384:768])
                for i in range(CB):
                    nc.sync.dma_start(WkvTs[i][:, 1152:1536],
                                      WkvT[128 * i:128 * (i + 1), 1152:1536])
                for h in range(H):
                    nc.sync.dma_start(Wp[h][0:96, :],
                                      WpjT[96 * h:96 * (h + 1), :])
                nc.sync.dma_start(Wp[H - 1][96:97, :], bias[:])

                ones_stage = pn.tile([128, 8], BF16, tag="ones")
                nc.vector.memset(ones_stage[:], 1.0)
                for nb in range(NB):
                    ones_cols = V[nb][:].rearrange(
                        "p (h c) -> p h c", h=H)[:, :, 96:97]
                    nc.vector.tensor_copy(ones_cols, ones_stage[:])

                W1 = wave_subwaves(
                    [(r, 0) for r in range(8)],
                    lambda g, cb: WqTs[cb][:, 96 * g[0]:96 * (g[0] + 1)],
                    lambda g, cb: xTs[cb][:, 0:512],
                    q_evac, D, 512)
                W2 = wave_subwaves(
                    [(h, u) for h in range(4) for u in range(2)],
                    lambda g, cb: WkvTs[cb][:, 96 * g[0]:96 * (g[0] + 1)],
                    lambda g, cb: kvTs[cb][:, 512 * g[1]:512 * (g[1] + 1)],
                    k_evac, D, 512)
                W3 = wave_subwaves(
                    [(h, u) for h in range(4, 8) for u in range(2)],
                    lambda g, cb: WkvTs[cb][:, 96 * g[0]:96 * (g[0] + 1)],
                    lambda g, cb: kvTs[cb][:, 512 * g[1]:512 * (g[1] + 1)],
                    k_evac, D, 512)
                W4 = wave_subwaves(
                    [(r, 1) for r in range(8)],
                    lambda g, cb: WqTs[cb][:, 96 * g[0]:96 * (g[0] + 1)],
                    lambda g, cb: xTs[cb][:, 512:1024],
                    q_evac, D, 512)
                W5 = wave_subwaves(
                    [(nb, 0) for nb in range(NB)],
                    lambda g, cb: kvTs[cb][:, 128 * g[0]:128 * (g[0] + 1)],
                    lambda g, cb: WkvTs[cb][:, C:C + 384],
                    v_evac, 128, 384)
                W6 = wave_subwaves(
                    [(nb, 1) for nb in range(NB)],
                    lambda g, cb: kvTs[cb][:, 128 * g[0]:128 * (g[0] + 1)],
                    lambda g, cb: WkvTs[cb][:, C + 384:C + 768],
                    v_evac, 128, 384)

                # direct: W1 + first K sub-wave (head 0) so S(0) can start
                for sw in W1:
                    for u in sw:
                        u()
                for u in W2[0]:
                    u()

                bulk = BulkStream()
                bulk.add(W2[1:])   # 21 units  -> done @21
                bulk.add(W5)       # 28        -> done @49
                bulk.add(W4)       # 28        -> done @77
                bulk.add(W3[0:1])  # 7         -> done @84
                bulk.add(W3[1:])   # 21        -> done @105
                bulk.add(W6)       # 28        -> done @133

                # interleave: S heads early to feed ACT; PV frees P tiles;
                # bulk positions gate deps (see docstring).
                for kb in range(NB):            # S0 (needs W2[0])
                    S_unit(0, kb)
                    bulk.emit(1)                # @8
                for kb in range(NB):            # S1 (W2[1] @7)
                    S_unit(1, kb)
                    bulk.emit(2)                # @24
                for kb in range(NB):            # S2 (W2[2] @14)
                    S_unit(2, kb)
                    bulk.emit(2)                # @40
                for kb in range(NB):            # S3 (W2[3] @21)
                    S_unit(3, kb)
                    bulk.emit(2)                # @56
                for kb in range(NB):            # PV0 (W5 @49, exp S0 done)
                    PV_unit(0, kb)
                    bulk.emit(2)                # @72
                emit_norm(0)
                for kb in range(NB):            # PV1
                    PV_unit(1, kb)
                    bulk.emit(2)                # @88
                emit_norm(1)
                for kb in range(NB):            # S4 (W4 @77 + W3[0] @84)
                    S_unit(4, kb)
                    bulk.emit(2)                # @104
                for kb in range(NB):            # PV2
                    PV_unit(2, kb)
                    bulk.emit(1)                # @112
                emit_norm(2)
                for kb in range(NB):            # S5 (W3[1] @91)
                    S_unit(5, kb)
                    bulk.emit(1)                # @120
                for kb in range(NB):            # PV3 + S6 (W3[2] @98)
                    PV_unit(3, kb)
                    S_unit(6, kb)
                    bulk.emit(1)                # @128
                emit_norm(3)
                bulk.drain()                    # W6 leftovers

              # pwq closed: xTs/WqTs freed.
              if True:
                def ypsum_slots(nb):
                    """Alternate y-psum groups between pW and pS tiles."""
                    t = (pW if nb % 2 == 0 else pS).tile(
                        [128, 1024], F32, tag="mm" if nb % 2 == 0 else "s",
                        name=f"yps{nb}")
                    return [t[0:128, 0:384], t[0:128, 512:896]]

                def yprojA_unit(nb):
                    """Heads 0-3 -> Ybuf[nb] (bf16, no DMA). Uses pW only:
                    pS is still cycled by S7 units in the same loop."""
                    t = pW.tile([128, 1024], F32, tag="mm", name=f"ypsA{nb}")
                    slots = [t[0:128, 0:384], t[0:128, 512:896]]
                    for u in range(2):
                        for i, h in enumerate([0, 1, 2, 3]):
                            nc.tensor.matmul(
                                slots[u],
                                Oall[0:96, 1024 * h + 128 * nb:
                                     1024 * h + 128 * (nb + 1)],
                                Wp[h][0:96, 384 * u:384 * (u + 1)],
                                start=(i == 0), stop=(i == 3))
                        nc.vector.tensor_copy(
                            Ybuf[nb][:, 384 * u:384 * (u + 1)], slots[u])

                def yprojB_mm(nb):
                    """Heads 4-7 (+bias row) into fresh psum slots."""
                    slots = ypsum_slots(nb)
                    for u in range(2):
                        for i, h in enumerate([4, 5, 6, 7]):
                            rows = 97 if h == H - 1 else 96
                            nc.tensor.matmul(
                                slots[u],
                                Oall[0:rows, 1024 * h + 128 * nb:
                                     1024 * h + 128 * (nb + 1)],
                                Wp[h][0:rows, 384 * u:384 * (u + 1)],
                                start=(i == 0), stop=(i == 3))
                    return slots

                def yprojB_fin(nb, slots):
                    """ysb = Ybuf[nb] + heads 4-7 psum; single y write."""
                    ysb = py.tile([128, C], F32, tag="ysb", name=f"ysb{nb}")
                    for u in range(2):
                        nc.vector.tensor_add(
                            ysb[:, 384 * u:384 * (u + 1)],
                            Ybuf[nb][:, 384 * u:384 * (u + 1)], slots[u])
                    nc.sync.dma_start(y[128 * nb:128 * (nb + 1), :], ysb[:])

                # PV4 + S7 interleaved with yprojA (heads 0-3)
                for i in range(NB):
                    PV_unit(4, i)
                    S_unit(7, i)                # (W3[3] drained above)
                    if i == NB - 1:
                        emit_norm(4)
                    yprojA_unit(i)
                for kb in range(NB):
                    PV_unit(5, kb)
                emit_norm_dve(5)
                for kb in range(NB):
                    PV_unit(6, kb)
                emit_norm_dve(6)
                for kb in range(NB):
                    PV_unit(7, kb)
                emit_norm_dve(7)
                # heads 4-7 + merge with Ybuf, two-deep psum pipeline
                prev = None
                for nb in range(NB):
                    slots = yprojB_mm(nb)
                    if prev is not None:
                        yprojB_fin(nb - 1, prev)
                    prev = slots
                yprojB_fin(NB - 1, prev)

    _legalize_waits(nc)
    return nc


def make_in_maps(x, kv, Wq, Wkv, Wproj, bproj):
    bf16 = mybir.dt.np(BF16)
    x = np.asarray(x, dtype=np.float32)
    kv = np.asarray(kv, dtype=np.float32)
    Wq = np.asarray(Wq, dtype=np.float32)
    Wkv = np.asarray(Wkv, dtype=np.float32)
    Wproj = np.asarray(Wproj, dtype=np.float32)
    bproj = np.asarray(bproj, dtype=np.float32)

    WqTs = (np.ascontiguousarray(Wq.T) * np.float32(SCALE)).astype(bf16)
    WkvT = np.ascontiguousarray(Wkv.T).astype(bf16)
    WpjT = np.ascontiguousarray(Wproj.T).astype(bf16)
    bias_np = np.ascontiguousarray(bproj.reshape(1, C)).astype(bf16)

    in_maps = []
    for b in range(B):
        in_maps.append({
            "xT": np.ascontiguousarray(x[b].T).astype(bf16),
            "kvT": np.ascontiguousarray(kv[b].T).astype(bf16),
            "WqT": WqTs,
            "WkvT": WkvT,
            "WpjT": WpjT,
            "bias": bias_np,
        })
    return in_maps


_NC_CACHE = {}


def kernel(x, kv, Wq, Wkv, Wproj, bproj, _trace=False):
    in_maps = make_in_maps(x, kv, Wq, Wkv, Wproj, bproj)
    if "nc" not in _NC_CACHE:
        _NC_CACHE["nc"] = build_kernel()
    nc = _NC_CACHE["nc"]
    res = run_bass_kernel_spmd(nc, in_maps, core_ids=list(range(B)),
                               trace=_trace)
    out = np.stack([r["y"] for r in res.results]).astype(np.float32)
    if _trace:
        return out, res
    return out


# revision 34
# speedup vs baseline: 3.4396x; 3.4396x over previous
"""CrossAttention Trainium2 Bass kernel (v2: phase-overlapped bf16 schedule).

Problem: y = CrossAttention(x, kv) with the reference's no-transpose q-reshape
quirk, B=8, N=1024, C=768, H=8, D=96.

Strategy: pure data parallelism — batch element b on NeuronCore b. Host
pre-transposes x/kv/weights, converts them to bf16 (halves input DMA and
SBUF; matmul rate is identical to float32r at 1 row/cycle; emulated end-to-end
rel err 0.6% vs the 2e-2 gate). All on-chip intermediates are bf16 with fp32
PSUM accumulation.

The ACT engine's ~68us of softmax-exp is the attention bottleneck, so the
schedule starts S=K^T Q tiles as soon as head 0's K/Q slices exist (~12us)
and spreads the remaining projection waves / PV / output-projection work
between S emissions so PE never starves while ACT drains. The output
projection avoids DRAM accumulation: heads 0-3 are staged into a bf16 SBUF
buffer (Ybuf) during the PV(4) region, heads 4-7 run as one PSUM pass at the
tail and merge with Ybuf via a single DVE add before one y write per n-block.
Row-sum normalization uses a DMA-round-trip broadcast for heads 0-4 and a
lower-latency DVE-reciprocal + ones-matmul broadcast for heads 5-7 (their
latency sits on the critical tail). Note: the tile scheduler reorders
instructions by dependencies, so emission order here only encodes
write-before-read legality and pool-recycle pressure, not the final schedule.

PSUM: pS 4 banks (S tiles + tail y-psum + norm broadcasts), pPV 2 banks (PV
accumulators), pW 2 banks (projection 2-group sub-waves + yprojA/tail).
"""
import sys
sys.path.insert(0, '/opt/trn_rl_repo')

import numpy as np
import concourse.bass as bass
import concourse.mybir as mybir
import concourse.tile as tile
from concourse.bass_utils import run_bass_kernel_spmd

F32 = mybir.dt.float32
BF16 = mybir.dt.bfloat16
AF = mybir.ActivationFunctionType

B, N, C = 8, 1024, 768
H, D = 8, 96
SCALE = D ** -0.5
NB = N // 128   # 8 n-blocks
CB = C // 128   # 6 c-blocks
HN = H * N      # 8192


def _legalize_waits(nc, max_waits=1):
    """This container's walrus accepts at most one sync-wait command per
    instruction; move excess waits onto preceding NoOps on the same engine."""
    ctr = 0
    for f in nc.m.functions:
        for blk in f.blocks:
            out = []
            changed = False
            for ins in blk.instructions:
                si = ins.sync_info
                waits = list(si.on_wait) if si is not None and si.on_wait else []
                if len(waits) > max_waits:
                    changed = True
                    for w in waits[:-max_waits]:
                        ctr += 1
                        nop = mybir.InstNoOp(name=f"I-wsplit-{ctr}")
                        nop.engine = ins.engine
                        nop.sync_info = mybir.SyncInfo(on_wait=[w], on_update=[])
                        out.append(nop)
                    ins.sync_info = mybir.SyncInfo(
                        on_wait=waits[-max_waits:],
                        on_update=list(si.on_update or []))
                out.append(ins)
            if changed:
                blk.instructions = out
    return ctr


def build_kernel(repeat=1):
    nc = bass.Bass('TRN2', target_bir_lowering=False, debug=False, num_devices=B)

    xT = nc.dram_tensor("xT", [C, N], BF16, kind="ExternalInput").ap()
    kvT = nc.dram_tensor("kvT", [C, N], BF16, kind="ExternalInput").ap()
    WqT = nc.dram_tensor("WqT", [C, C], BF16, kind="ExternalInput").ap()
    WkvT = nc.dram_tensor("WkvT", [C, 2 * C], BF16, kind="ExternalInput").ap()
    WpjT = nc.dram_tensor("WpjT", [C, C], BF16, kind="ExternalInput").ap()
    bias = nc.dram_tensor("bias", [1, C], BF16, kind="ExternalInput").ap()
    y = nc.dram_tensor("y", [N, C], F32, kind="ExternalOutput").ap()
    rs_dram = nc.dram_tensor("rs_scratch", [1, HN], BF16, kind="Internal").ap()
    ri_dram = nc.dram_tensor("ri_scratch", [1, HN], BF16, kind="Internal").ap()

    with tile.TileContext(nc) as tc:
      for _rep in range(repeat):
        with tc.tile_pool(name="persist", bufs=1) as pp, \
             tc.tile_pool(name="norm", bufs=1) as pn, \
             tc.tile_pool(name="ptile", bufs=25) as ppt, \
             tc.tile_pool(name="yout", bufs=3) as py, \
             tc.tile_pool(name="wkv", bufs=1) as pwkv, \
             tc.tile_pool(name="psS", bufs=2, space="PSUM") as pS, \
             tc.tile_pool(name="psPV", bufs=2, space="PSUM") as pPV, \
             tc.tile_pool(name="psW", bufs=1, space="PSUM") as pW:
            QT = pp.tile([D, HN], BF16, tag="QT")
            KT = pp.tile([D, HN], BF16, tag="KT")
            V = [pp.tile([128, H * 97], BF16, tag=f"V{i}", name=f"V{i}")
                 for i in range(NB)]
            Oall = pp.tile([97, HN], BF16, tag="Oall")
            # partial y (heads 0-3) per n-block, bf16 accumulator staging
            Ybuf = [pp.tile([128, C], BF16, tag=f"Yb{i}", name=f"Ybuf{i}")
                    for i in range(NB)]

            ones97f = pn.tile([1, 97], F32, tag="o97f")
            nc.vector.memset(ones97f[:], 1.0)
            ones97 = pn.tile([1, 97], BF16, tag="o97")
            nc.vector.tensor_copy(ones97[:], ones97f[:])

            kvTs = [pwkv.tile([128, N], BF16, tag=f"kv{i}", name=f"kvTs{i}")
                    for i in range(CB)]
            WkvTs = [pwkv.tile([128, 2 * C], BF16, tag=f"Wkv{i}",
                               name=f"WkvTs{i}") for i in range(CB)]

            # ---------------- helpers ----------------
            def q_evac(g, ps):
                r, u = g
                dest = QT[:].rearrange(
                    "p (h j r) -> p h j r", h=H, j=128)[
                    :, 4 * u:4 * (u + 1), :, r:r + 1]
                nc.vector.tensor_copy(dest, ps)

            def k_evac(g, ps):
                h, u = g
                nc.vector.tensor_copy(
                    KT[:, 1024 * h + 512 * u:1024 * h + 512 * (u + 1)], ps)

            def v_evac(g, ps):
                nb, u = g
                dest = V[nb][:].rearrange(
                    "p (h c) -> p h c", h=H)[:, 4 * u:4 * (u + 1), 0:96]
                nc.vector.tensor_copy(dest, ps)

            _sw = [0]

            def subwave_units(groups, lhsT_of, rhs_of, evac, mm_parts, ncols):
                """One 2-group sub-wave on pW -> list of emit closures
                (6 cb-steps + 1 evac). The psum tile is allocated when the
                first closure runs."""
                assert len(groups) == 2
                state = {}

                def step(cb):
                    if cb == 0:
                        _sw[0] += 1
                        t = pW.tile([128, 1024], F32, tag="mm",
                                    name=f"sw{_sw[0]}")
                        state['slots'] = [t[0:mm_parts, 0:ncols],
                                          t[0:mm_parts, 512:512 + ncols]]
                    for g, ps in zip(groups, state['slots']):
                        nc.tensor.matmul(
                            ps, lhsT_of(g, cb), rhs_of(g, cb),
                            start=(cb == 0), stop=(cb == CB - 1))

                def final():
                    for g, ps in zip(groups, state['slots']):
                        evac(g, ps)
                return [lambda cb=cb: step(cb) for cb in range(CB)] + [final]

            def wave_subwaves(groups, *a):
                return [subwave_units(groups[i:i + 2], *a)
                        for i in range(0, len(groups), 2)]

            class BulkStream:
                def __init__(self):
                    self.units = []

                def add(self, subwaves):
                    for sw in subwaves:
                        self.units.extend(sw)

                def emit(self, n=1):
                    for _ in range(n):
                        if self.units:
                            self.units.pop(0)()

                def drain(self):
                    while self.units:
                        self.units.pop(0)()

            P_tiles = {h: [None] * NB for h in range(H)}

            def S_unit(h, kb):
                pt = ppt.tile([128, N], BF16, tag="pt", name=f"P{h}_{kb}")
                ps = pS.tile([128, 1024], F32, tag="s", name=f"s{h}_{kb}")
                for u in range(2):
                    nc.tensor.matmul(
                        ps[:, 512 * u:512 * (u + 1)],
                        KT[:, 1024 * h + 128 * kb:1024 * h + 128 * (kb + 1)],
                        QT[:, 1024 * h + 512 * u:1024 * h + 512 * (u + 1)],
                        start=True, stop=True)
                nc.scalar.activation(pt[:], ps[:], AF.Exp)
                P_tiles[h][kb] = pt

            pv_state = {}

            def PV_unit(h, kb):
                if kb == 0:
                    pv_state[h] = [pPV.tile([97, 512], F32, tag="po",
                                            name=f"po{h}_{u}")
                                   for u in range(2)]
                for u in range(2):
                    nc.tensor.matmul(
                        pv_state[h][u][:],
                        V[kb][:, 97 * h:97 * (h + 1)],
                        P_tiles[h][kb][:, 512 * u:512 * (u + 1)],
                        start=(kb == 0), stop=(kb == NB - 1))
                if kb == NB - 1:
                    for u in range(2):
                        nc.vector.tensor_copy(
                            Oall[:, 1024 * h + 512 * u:
                                 1024 * h + 512 * (u + 1)],
                            pv_state[h][u][:])
                    P_tiles[h] = None

            def emit_norm(h):
                """rowsum -> 1/rowsum broadcast (DMA round trip) ->
                in-place normalize Oall's head-h slice."""
                sl = slice(1024 * h, 1024 * (h + 1))
                nc.sync.dma_start(rs_dram[0:1, sl], Oall[96:97, sl])
                rsh = pn.tile([128, 8], BF16, tag="rs", name=f"rs{h}", bufs=2)
                nc.sync.dma_start(
                    rsh[:],
                    rs_dram[0:1, sl].rearrange("p (a b) -> (p a) b", a=128))
                rih = pn.tile([128, 8], F32, tag="ri", name=f"ri{h}", bufs=2)
                nc.vector.reciprocal(rih[:], rsh[:])
                rirh = pn.tile([128, 8], BF16, tag="rir", name=f"rir{h}",
                               bufs=2)
                nc.vector.tensor_copy(rirh[:], rih[:])
                nc.sync.dma_start(
                    ri_dram[0:1, sl].rearrange("p (a b) -> (p a) b", a=128),
                    rirh[:])
                bch = pn.tile([97, N], BF16, tag="bc", name=f"bc{h}", bufs=2)
                nc.sync.dma_start(
                    bch[:], bass.AP(ri_dram.tensor, 1024 * h, [[0, 97], [1, N]]))
                nc.vector.tensor_mul(Oall[:, sl], Oall[:, sl], bch[:])

            def emit_norm_dve(h):
                """Lowest-latency norm: reciprocal of the rowsum row on DVE
                (single partition), broadcast via K=1 ones matmul on PE.
                No DRAM round trip, no ACT table dependency."""
                sl = slice(1024 * h, 1024 * (h + 1))
                invt = pn.tile([1, N], BF16, tag="invr2", name=f"inv2_{h}",
                               bufs=2)
                with nc.allow_low_precision(reason="1/rowsum broadcast scale"):
                    nc.vector.reciprocal(invt[:], Oall[96:97, sl])
                bct = pS.tile([128, 1024], F32, tag="s",
                              name=f"bcd{h}")
                for u in range(2):
                    bc_ps = bct[0:97, 512 * u:512 * (u + 1)]
                    nc.tensor.matmul(
                        bc_ps, ones97[:],
                        invt[0:1, 512 * u:512 * (u + 1)],
                        start=True, stop=True)
                    ssl = slice(1024 * h + 512 * u, 1024 * h + 512 * (u + 1))
                    nc.vector.tensor_mul(Oall[:, ssl], Oall[:, ssl], bc_ps)

            def emit_norm_fast(h):
                """No-DMA tail variant: inv = exp(-ln(rowsum)) on ACT,
                broadcast via K=1 ones matmul on PE."""
                sl = slice(1024 * h, 1024 * (h + 1))
                lnr = pn.tile([1, N], F32, tag="lnx", name=f"lnr{h}")
                nc.scalar.activation(lnr[:], Oall[96:97, sl], AF.Ln)
                invt = pn.tile([1, N], BF16, tag="invr", name=f"invr{h}")
                nc.scalar.activation(invt[:], lnr[:], AF.Exp, scale=-1.0)
                for u in range(2):
                    bc_ps = pPV.tile([97, 512], F32, tag="po",
                                     name=f"bcps{h}_{u}")
                    nc.tensor.matmul(
                        bc_ps[:], ones97[:],
                        invt[0:1, 512 * u:512 * (u + 1)],
                        start=True, stop=True)
                    ssl = slice(1024 * h + 512 * u, 1024 * h + 512 * (u + 1))
                    nc.vector.tensor_mul(Oall[:, ssl], Oall[:, ssl], bc_ps[:])

            # ---------------- phase A+B ----------------
            with tc.tile_pool(name="wproj", bufs=1) as pwp:
              Wp = []
              for h in range(H):
                  rows = 97 if h == H - 1 else 96
                  Wp.append(pwp.tile([rows, C], BF16, tag=f"Wp{h}",
                                     name=f"Wp{h}"))
              with tc.tile_pool(name="wq", bufs=1) as pwq:
                xTs = [pwq.tile([128, N], BF16, tag=f"xT{i}", name=f"xTs{i}")
                       for i in range(CB)]
                WqTs = [pwq.tile([128, C], BF16, tag=f"Wq{i}", name=f"WqTs{i}")
                        for i in range(CB)]

                # DMA issue in consumption order:
                # W1 (Wq + x half0), W2 (kv + Wkv K-half lo), W5 (Wkv V lo),
                # W4 (x half1), W3 (Wkv K-half hi), W6 (Wkv V hi), Wp+bias.
                nc.sync.dma_start(WqTs[0][:, 0:192], WqT[0:128, 0:192])
                nc.sync.dma_start(xTs[0][:, 0:512], xT[0:128, 0:512])
                nc.sync.dma_start(WqTs[0][:, 192:C], WqT[0:128, 192:C])
                for i in range(1, CB):
                    nc.sync.dma_start(WqTs[i][:], WqT[128 * i:128 * (i + 1), :])
                    nc.sync.dma_start(xTs[i][:, 0:512],
                                      xT[128 * i:128 * (i + 1), 0:512])
                for i in range(CB):
                    nc.sync.dma_start(kvTs[i][:], kvT[128 * i:128 * (i + 1), :])
                    nc.sync.dma_start(WkvTs[i][:, 0:384],
                                      WkvT[128 * i:128 * (i + 1), 0:384])
                for i in range(CB):
                    nc.sync.dma_start(WkvTs[i][:, 768:1152],
                                      WkvT[128 * i:128 * (i + 1), 768:1152])
                for i in range(CB):
                    nc.sync.dma_start(xTs[i][:, 512:1024],
                                      xT[128 * i:128 * (i + 1), 512:1024])
                for i in range(CB):
                    nc.sync.dma_start(WkvTs[i][:, 384:768],
                                      WkvT[128 * i:128 * (i + 1), 384:768])
                for i in range(CB):
                    nc.sync.dma_start(WkvTs[i][:, 1152:1536],
                                      WkvT[128 * i:128 * (i + 1), 1152:1536])
                for h in range(H):
                    nc.sync.dma_start(Wp[h][0:96, :],
                                      WpjT[96 * h:96 * (h + 1), :])
                nc.sync.dma_start(Wp[H - 1][96:97, :], bias[:])

                ones_stage = pn.tile([128, 8], BF16, tag="ones")
                nc.vector.memset(ones_stage[:], 1.0)
                for nb in range(NB):
                    ones_cols = V[nb][:].rearrange(
                        "p (h c) -> p h c", h=H)[:, :, 96:97]
                    nc.vector.tensor_copy(ones_cols, ones_stage[:])

                W1 = wave_subwaves(
                    [(r, 0) for r in range(8)],
                    lambda g, cb: WqTs[cb][:, 96 * g[0]:96 * (g[0] + 1)],
                    lambda g, cb: xTs[cb][:, 0:512],
                    q_evac, D, 512)
                W2 = wave_subwaves(
                    [(h, u) for h in range(4) for u in range(2)],
                    lambda g, cb: WkvTs[cb][:, 96 * g[0]:96 * (g[0] + 1)],
                    lambda g, cb: kvTs[cb][:, 512 * g[1]:512 * (g[1] + 1)],
                    k_evac, D, 512)
                W3 = wave_subwaves(
                    [(h, u) for h in range(4, 8) for u in range(2)],
                    lambda g, cb: WkvTs[cb][:, 96 * g[0]:96 * (g[0] + 1)],
                    lambda g, cb: kvTs[cb][:, 512 * g[1]:512 * (g[1] + 1)],
                    k_evac, D, 512)
                W4 = wave_subwaves(
                    [(r, 1) for r in range(8)],
                    lambda g, cb: WqTs[cb][:, 96 * g[0]:96 * (g[0] + 1)],
                    lambda g, cb: xTs[cb][:, 512:1024],
                    q_evac, D, 512)
                W5 = wave_subwaves(
                    [(nb, 0) for nb in range(NB)],
                    lambda g, cb: kvTs[cb][:, 128 * g[0]:128 * (g[0] + 1)],
                    lambda g, cb: WkvTs[cb][:, C:C + 384],
                    v_evac, 128, 384)
                W6 = wave_subwaves(
                    [(nb, 1) for nb in range(NB)],
                    lambda g, cb: kvTs[cb][:, 128 * g[0]:128 * (g[0] + 1)],
                    lambda g, cb: WkvTs[cb][:, C + 384:C + 768],
                    v_evac, 128, 384)

                # direct: W1 + first K sub-wave (head 0) so S(0) can start
                for sw in W1:
                    for u in sw:
                        u()
                for u in W2[0]:
                    u()

                bulk = BulkStream()
                bulk.add(W2[1:])   # 21 units  -> done @21
                bulk.add(W5)       # 28        -> done @49
                bulk.add(W4)       # 28        -> done @77
                bulk.add(W3[0:1])  # 7         -> done @84
                bulk.add(W3[1:])   # 21        -> done @105
                bulk.add(W6)       # 28        -> done @133

                # interleave: S heads early to feed ACT; PV frees P tiles;
                # bulk positions gate deps (see docstring).
                for kb in range(NB):            # S0 (needs W2[0])
                    S_unit(0, kb)
                    bulk.emit(1)                # @8
                for kb in range(NB):            # S1 (W2[1] @7)
                    S_unit(1, kb)
                    bulk.emit(2)                # @24
                for kb in range(NB):            # S2 (W2[2] @14)
                    S_unit(2, kb)
                    bulk.emit(2)                # @40
                for kb in range(NB):            # S3 (W2[3] @21)
                    S_unit(3, kb)
                    bulk.emit(2)                # @56
                for kb in range(NB):            # PV0 (W5 @49, exp S0 done)
                    PV_unit(0, kb)
                    bulk.emit(2)                # @72
                emit_norm(0)
                for kb in range(NB):            # PV1
                    PV_unit(1, kb)
                    bulk.emit(2)                # @88
                emit_norm(1)
                for kb in range(NB):            # S4 (W4 @77 + W3[0] @84)
                    S_unit(4, kb)
                    bulk.emit(2)                # @104
                for kb in range(NB):            # PV2
                    PV_unit(2, kb)
                    bulk.emit(1)                # @112
                emit_norm(2)
                for kb in range(NB):            # S5 (W3[1] @91)
                    S_unit(5, kb)
                    bulk.emit(1)                # @120
                for kb in range(NB):            # PV3 + S6 (W3[2] @98)
                    PV_unit(3, kb)
                    S_unit(6, kb)
                    bulk.emit(1)                # @128
                emit_norm(3)
                bulk.drain()                    # W6 leftovers

              # pwq closed: xTs/WqTs freed.
              if True:
                def ypsum_slots(nb):
                    """Alternate y-psum groups between pW and pS tiles."""
                    t = (pW if nb % 2 == 0 else pS).tile(
                        [128, 1024], F32, tag="mm" if nb % 2 == 0 else "s",
                        name=f"yps{nb}")
                    return [t[0:128, 0:384], t[0:128, 512:896]]

                def yprojA_unit(nb):
                    """Heads 0-3 -> Ybuf[nb] (bf16, no DMA). Uses pW only:
                    pS is still cycled by S7 units in the same loop."""
                    t = pW.tile([128, 1024], F32, tag="mm", name=f"ypsA{nb}")
                    slots = [t[0:128, 0:384], t[0:128, 512:896]]
                    for u in range(2):
                        for i, h in enumerate([0, 1, 2, 3]):
                            nc.tensor.matmul(
                                slots[u],
                                Oall[0:96, 1024 * h + 128 * nb:
                                     1024 * h + 128 * (nb + 1)],
                                Wp[h][0:96, 384 * u:384 * (u + 1)],
                                start=(i == 0), stop=(i == 3))
                        nc.vector.tensor_copy(
                            Ybuf[nb][:, 384 * u:384 * (u + 1)], slots[u])

                def yprojB_mm(nb):
                    """Heads 4-7 (+bias row) into fresh psum slots."""
                    slots = ypsum_slots(nb)
                    for u in range(2):
                        for i, h in enumerate([4, 5, 6, 7]):
                            rows = 97 if h == H - 1 else 96
                            nc.tensor.matmul(
                                slots[u],
                                Oall[0:rows, 1024 * h + 128 * nb:
                                     1024 * h + 128 * (nb + 1)],
                                Wp[h][0:rows, 384 * u:384 * (u + 1)],
                                start=(i == 0), stop=(i == 3))
                    return slots

                def yprojB_fin(nb, slots):
                    """ysb = Ybuf[nb] + heads 4-7 psum; single y write."""
                    ysb = py.tile([128, C], F32, tag="ysb", name=f"ysb{nb}")
                    for u in range(2):
                        nc.vector.tensor_add(
                            ysb[:, 384 * u:384 * (u + 1)],
                            Ybuf[nb][:, 384 * u:384 * (u + 1)], slots[u])
                    nc.sync.dma_start(y[128 * nb:128 * (nb + 1), :], ysb[:])

                # PV4 + S7 interleaved with yprojA (heads 0-3)
                for i in range(NB):
                    PV_unit(4, i)
                    S_unit(7, i)                # (W3[3] drained above)
                    if i == NB - 1:
                        emit_norm(4)
                    yprojA_unit(i)
                for kb in range(NB):
                    PV_unit(5, kb)
                emit_norm_dve(5)
                for kb in range(NB):
                    PV_unit(6, kb)
                emit_norm_dve(6)
                for kb in range(NB):
                    PV_unit(7, kb)
                emit_norm_dve(7)
                # heads 4-7 + merge with Ybuf, two-deep psum pipeline
                prev = None
                for nb in range(NB):
                    slots = yprojB_mm(nb)
                    if prev is not None:
                        yprojB_fin(nb - 1, prev)
                    prev = slots
                yprojB_fin(NB - 1, prev)

    _legalize_waits(nc)
    return nc


def make_in_maps(x, kv, Wq, Wkv, Wproj, bproj):
    bf16 = mybir.dt.np(BF16)
    x = np.asarray(x, dtype=np.float32)
    kv = np.asarray(kv, dtype=np.float32)
    Wq = np.asarray(Wq, dtype=np.float32)
    Wkv = np.asarray(Wkv, dtype=np.float32)
    Wproj = np.asarray(Wproj, dtype=np.float32)
    bproj = np.asarray(bproj, dtype=np.float32)

    WqTs = (np.ascontiguousarray(Wq.T) * np.float32(SCALE)).astype(bf16)
    WkvT = np.ascontiguousarray(Wkv.T).astype(bf16)
    WpjT = np.ascontiguousarray(Wproj.T).astype(bf16)
    bias_np = np.ascontiguousarray(bproj.reshape(1, C)).astype(bf16)

    in_maps = []
    for b in range(B):
        in_maps.append({
            "xT": np.ascontiguousarray(x[b].T).astype(bf16),
            "kvT": np.ascontiguousarray(kv[b].T).astype(bf16),
            "WqT": WqTs,
            "WkvT": WkvT,
            "WpjT": WpjT,
            "bias": bias_np,
        })
    return in_maps


_NC_CACHE = {}


def kernel(x, kv, Wq, Wkv, Wproj, bproj, _trace=False):
    in_maps = make_in_maps(x, kv, Wq, Wkv, Wproj, bproj)
    if "nc" not in _NC_CACHE:
        _NC_CACHE["nc"] = build_kernel()
    nc = _NC_CACHE["nc"]
    res = run_bass_kernel_spmd(nc, in_maps, core_ids=list(range(B)),
                               trace=_trace)
    out = np.stack([r["y"] for r in res.results]).astype(np.float32)
    if _trace:
        return out, res
    return out
